# revision 4
# baseline (speedup 1.0000x reference)
"""Trainium2 Bass kernel for multi-head attention (dense_transformer).

Full module: qkv = x @ W_qkv + b_qkv; multi-head attention (16 heads, d=64,
N=4096); out = attn @ W_proj + b_proj.

Sharding: tensor-parallel over heads - 2 heads per core on 8 cores. Each core
receives full x (pre-transposed on host to [C, N]) plus its head-slices of the
weights, computes its heads' attention and a partial output projection; the
host sums the 8 fp16 partials in f32 and adds b_proj plus the bv@Wp
correction (softmax rows sum to 1, so the V-bias contribution to the output
is the constant row bv @ Wp - computed on host, never on device).

v2 structure (per core, matmul operands fp16, PSUM f32):
  A) Q^T,K^T [128, 4096] = W^T @ x^T accumulated over C chunks, bias added on
     eviction. V^T likewise, PE-transposed to natural [tok, d] with a ones
     column per head: [V_h0 | 1 | V_h1 | 1] (cols 64/129 = denominator trick).
  B) per (q-chunk 512, k-chunk 128):
     - scores: TWO ROW-TILED matmuls (K=64 each, heads at PE rows 0-63 /
       64-127) run concurrently -> s [128, 1024] f32 (2 PSUM banks).
     - exp: split across engines. Most k-chunks: ACT exp (scale=1/8) PSUM ->
       fp16 SBUF. A spaced subset: custom 2-pass DVE op computing
       exp(s/8) = (T3(s/512))^64 (Taylor-3 + 6 squarings, max rel err 1.2e-4)
       so the Vector engine carries part of the 33.5M-element exp load that
       otherwise serializes on the Scalar engine (1 elem/lane/cycle).
     - AV: 2 matmuls (M=65, [V_h|1] stationary) accumulate av_h [65, 512];
       row 64 = softmax denominator. AV emission is deferred by a per-tile
       lag so slow DVE-exp tiles don't stall the PE.
     - normalize: DVE reciprocal of row 64 + GPSIMD partition broadcast +
       DVE mul into packed aoT [128, 4096] fp16 (h0 rows 0-63, h1 64-127).
  C) proj (lagged one q-chunk, spread through the ki loop): single matmul
     per chunk, K=128 contracts both heads at once: out[tok, C-half] =
     aoT_chunk^T @ Wp; evicted to fp16 and DMAd out.
"""

import numpy as np
from contextlib import ExitStack

NUM_CORES = 8
DIM = 1024
NUM_HEADS = 16
HDIM = 64
N = 4096
HPC = NUM_HEADS // NUM_CORES   # heads per core = 2
DPC = HPC * HDIM               # head dims per core = 128

# k-chunks (of 32 per q-chunk) whose exp runs on the DVE instead of ACT.
DVE_KIS = frozenset((14,))  # bisect: one DVE tile
LAG_ACT = 2   # AV follows scores by this many slots (ACT-exp tiles)
LAG_DVE = 5   # and for DVE-exp tiles (2-pass exp has ~2.3us latency)

_NC_CACHE = {}
_DVE_OPS = {}


def _register_dve_exp_ops():
    """Register the two custom DVE ops for exp(s/8) = (T3(s/512))^64.

    P1: x = s*C0; t = 1 + x + x^2/2 + x^3/6 (Horner); out = t^2   (8 stages)
    P2: out = in^32 (5 squarings)                                  (5 stages)
    """
    if _DVE_OPS:
        return _DVE_OPS
    from concourse.dve_spec import Spec, Src0, C0, C1, C2, One, sq, lower
    from concourse import dve_ops as dvo
    from concourse.dve_uop import DveOpSpec

    def _p1_ref(in0, in1, s0, s1, imm2):
        x = (np.asarray(in0, np.float32) * np.float32(s0)).astype(np.float32)
        h = (x * np.float32(s1)).astype(np.float32)
        h = (h + np.float32(imm2)).astype(np.float32)
        h = (h * x).astype(np.float32)
        h = (h + np.float32(1.0)).astype(np.float32)
        h = (h * x).astype(np.float32)
        h = (h + np.float32(1.0)).astype(np.float32)
        return (h * h).astype(np.float32)

    def _p2_ref(in0, in1, s0, s1, imm2):
        x = np.asarray(in0, np.float32)
        for _ in range(5):
            x = (x * x).astype(np.float32)
        return x

    x = Src0 * C0
    h = x * C1
    h = h + C2
    h = h * x
    h = h + One
    h = h * x
    h = h + One
    spec1 = Spec(body=sq(h), reference=_p1_ref)
    spec2 = Spec(body=sq(sq(sq(sq(sq(Src0))))), reference=_p2_ref)

    for name, spec in (("EXP_P1_ANT", spec1), ("EXP_P2_ANT", spec2)):
        if name in dvo._SUB_OPCODE_FOR_NAME:
            continue
        row = dvo._CUSTOM_DVE_ROW_BASE + len(dvo.OPS)
        assert row < 0x20
        shas = {}
        for ver in ("v3", "v4"):
            ds = DveOpSpec(name=name, opcode=row, uops=lower(spec, ver=ver),
                           rd1_en=False)
            shas[ver] = ds.sha(ver)
        op = dvo.DveOp(name, spec, subdim=False, uops_sha=shas)
        dvo.OPS.append(op)
        dvo._SUB_OPCODE_FOR_NAME[name] = row
        dvo.CUSTOM_DVE_SPECS[name] = spec
        _DVE_OPS[name] = op
    return _DVE_OPS


def build_nc(reps=1, trace_sim=False):
    if (reps, trace_sim) in _NC_CACHE:
        return _NC_CACHE[(reps, trace_sim)]

    import concourse.bass as bass
    import concourse.mybir as mybir
    import concourse.tile as tile
    from concourse import bacc
    from concourse.masks import make_identity

    ops = _register_dve_exp_ops()
    EXP_P1, EXP_P2 = ops["EXP_P1_ANT"], ops["EXP_P2_ANT"]

    f32 = mybir.dt.float32
    fp16 = mybir.dt.float16
    AF = mybir.ActivationFunctionType
    ts = bass.ts

    nc = bacc.Bacc(trn_type="TRN2", target_bir_lowering=False, debug=False)
    xT = nc.dram_tensor("xT", [DIM, N], fp16, kind="ExternalInput").ap()
    wq = nc.dram_tensor("wq", [DIM, DPC], fp16, kind="ExternalInput").ap()
    wk = nc.dram_tensor("wk", [DIM, DPC], fp16, kind="ExternalInput").ap()
    wv = nc.dram_tensor("wv", [DIM, DPC], fp16, kind="ExternalInput").ap()
    wp = nc.dram_tensor("wp", [DPC, DIM], fp16, kind="ExternalInput").ap()
    bq = nc.dram_tensor("bq", [DPC, 1], f32, kind="ExternalInput").ap()
    bk = nc.dram_tensor("bk", [DPC, 1], f32, kind="ExternalInput").ap()
    ones = nc.dram_tensor("ones", [1, 1], fp16, kind="ExternalInput").ap()
    out = nc.dram_tensor("out", [N, DIM], fp16, kind="ExternalOutput").ap()

    with tile.TileContext(nc, trace_sim=trace_sim) as tc, ExitStack() as ctx:
        singles = ctx.enter_context(tc.tile_pool(name="singles", bufs=1))
        psum = ctx.enter_context(tc.tile_pool(name="ps", bufs=2, space="PSUM"))
        xpool = ctx.enter_context(tc.tile_pool(name="xp", bufs=2))
        work = ctx.enter_context(tc.tile_pool(name="work", bufs=2))
        ppool = ctx.enter_context(tc.tile_pool(name="pp", bufs=3))
        opool = ctx.enter_context(tc.tile_pool(name="op", bufs=3))

        ident = singles.tile([128, 128], f32, tag="ident")
        make_identity(nc, ident)

        wq_sb = singles.tile([128, 8, DPC], fp16, tag="wq")
        wk_sb = singles.tile([128, 8, DPC], fp16, tag="wk")
        wv_sb = singles.tile([128, 8, DPC], fp16, tag="wv")
        nc.sync.dma_start(out=wq_sb, in_=wq.rearrange("(c p) m -> p c m", p=128))
        nc.sync.dma_start(out=wk_sb, in_=wk.rearrange("(c p) m -> p c m", p=128))
        nc.sync.dma_start(out=wv_sb, in_=wv.rearrange("(c p) m -> p c m", p=128))
        wp_sb = singles.tile([128, DIM], fp16, tag="wp")
        nc.sync.dma_start(out=wp_sb, in_=wp)
        bq_sb = singles.tile([DPC, 1], f32, tag="bq")
        bk_sb = singles.tile([DPC, 1], f32, tag="bk")
        nc.sync.dma_start(out=bq_sb, in_=bq)
        nc.sync.dma_start(out=bk_sb, in_=bk)

        qT = singles.tile([128, N], fp16, tag="qT")
        kT = singles.tile([128, N], fp16, tag="kT")
        aoT = singles.tile([128, N], fp16, tag="aoT")
        # V natural layout + ones column per head: [.., t, 0:64]=V_h0,
        # [.., t, 64]=1, [.., t, 65:129]=V_h1, [.., t, 129]=1
        v_nat = singles.tile([128, 32, 130], fp16, tag="vnat")
        nc.sync.dma_start(out=v_nat[:, :, 64:65], in_=ones.to_broadcast((128, 32, 1)))
        nc.sync.dma_start(out=v_nat[:, :, 129:130], in_=ones.to_broadcast((128, 32, 1)))

        for _rep in range(reps):
            # ---------------- Phase A: QKV projection ----------------
            for qt in range(4):
                xt = [xpool.tile([128, 1024], fp16, tag=f"x{c}", name=f"x{c}") for c in range(8)]
                for c in range(8):
                    nc.sync.dma_start(out=xt[c], in_=xT[ts(c, 128), ts(qt, 1024)])
                for nl in range(2):
                    n = qt * 2 + nl
                    # K and V first: attention waits on full K/V, while Q
                    # chunks are consumed per q-tile
                    acc = psum.tile([128, 512], f32, tag="pj", name="kacc", bufs=2)
                    for c in range(8):
                        nc.tensor.matmul(
                            acc, wk_sb[:, c, :], xt[c][:, ts(nl, 512)],
                            start=(c == 0), stop=(c == 7),
                        )
                    nc.vector.tensor_scalar_add(kT[:, ts(n, 512)], acc, bk_sb)
                    vacc = psum.tile([128, 512], f32, tag="pj", name="vacc", bufs=2)
                    for c in range(8):
                        nc.tensor.matmul(
                            vacc, wv_sb[:, c, :], xt[c][:, ts(nl, 512)],
                            start=(c == 0), stop=(c == 7),
                        )
                    vst = work.tile([128, 512], f32, tag="vst")
                    nc.scalar.copy(vst, vacc)
                    tpb = psum.tile([128, 512], f32, tag="s", name="tpb", bufs=2)
                    for tl in range(4):
                        nc.tensor.transpose(
                            tpb[:, ts(tl, 128)], vst[:, ts(tl, 128)], ident)
                    nc.vector.tensor_copy(
                        out=v_nat[:, ts(n, 4), 0:130]
                        .rearrange("p t (g d) -> p t g d", d=65)[:, :, :, 0:64],
                        in_=tpb.rearrange("p (t g d) -> p t g d", g=2, d=64),
                    )
                    qacc = psum.tile([128, 512], f32, tag="s", name="qacc", bufs=2)
                    for c in range(8):
                        nc.tensor.matmul(
                            qacc, wq_sb[:, c, :], xt[c][:, ts(nl, 512)],
                            start=(c == 0), stop=(c == 7),
                        )
                    nc.vector.tensor_scalar_add(qT[:, ts(n, 512)], qacc, bq_sb)

            # ---------------- Phase B: attention + lagged projection ------
            def emit_proj_chunk(t, j):
                pp = psum.tile([128, 512], f32, tag="pj", name="pp", bufs=2)
                nc.tensor.matmul(
                    pp, aoT[:, ts(t, 128)], wp_sb[:, ts(j, 512)],
                    start=True, stop=True,
                )
                ot = opool.tile([128, 512], fp16, tag="ot")
                nc.vector.tensor_copy(ot, pp)
                nc.sync.dma_start(out=out[ts(t, 128), ts(j, 512)], in_=ot)

            for qi in range(8):
                # proj tasks for the previous q-chunk, spread across this
                # q-chunk's slot loop so they fill PE slack
                proj_tasks = (
                    [((qi - 1) * 4 + tl, j) for tl in range(4) for j in range(2)]
                    if qi >= 1 else []
                )
                av = [
                    psum.tile([65, 512], f32, tag="av0", name="av0", bufs=1),
                    psum.tile([65, 512], f32, tag="av1", name="av1", bufs=1),
                ]
                pending = []  # (ki, p_tile, due_slot)
                n_av = 0
                for t in range(32 + LAG_DVE + 1):
                    if t < 32:
                        ki = t
                        s = psum.tile([128, 1024], f32, tag="s", name="s", bufs=2)
                        # two row-tiled matmuls: head0 on PE rows 0-63,
                        # head1 on rows 64-127 - run concurrently
                        nc.tensor.matmul(
                            s[:, 0:512], kT[0:64, ts(ki, 128)], qT[0:64, ts(qi, 512)],
                            start=True, stop=True,
                        )
                        nc.tensor.matmul(
                            s[:, 512:1024], kT[64:128, ts(ki, 128)],
                            qT[64:128, ts(qi, 512)],
                            start=True, stop=True,
                        )
                        p = ppool.tile([128, 1024], fp16, tag="p")
                        if ki in DVE_KIS:
                            mid = work.tile([128, 1024], f32, tag="mid", bufs=2)
                            nc.vector._custom_dve(
                                EXP_P1, out=mid, in0=s,
                                s0=1.0 / 512.0, s1=1.0 / 6.0, imm2=0.5)
                            nc.vector._custom_dve(EXP_P2, out=p, in0=mid)
                            due = t + LAG_DVE
                        else:
                            nc.scalar.activation(p, s, AF.Exp, scale=0.125)
                            due = t + LAG_ACT
                        pending.append((ki, p, due))
                    ready = [e for e in pending if e[2] <= t]
                    for e in ready:
                        pending.remove(e)
                        ki, p, _ = e
                        nc.tensor.matmul(
                            av[0], v_nat[:, ki, 0:65], p[:, 0:512],
                            start=(n_av == 0), stop=(n_av == 31),
                        )
                        nc.tensor.matmul(
                            av[1], v_nat[:, ki, 65:130], p[:, 512:1024],
                            start=(n_av == 0), stop=(n_av == 31),
                        )
                        n_av += 1
                    if t % 4 == 2 and proj_tasks:
                        emit_proj_chunk(*proj_tasks.pop(0))
                assert n_av == 32 and not pending
                for h in range(2):
                    recip = work.tile([1, 512], f32, tag="recip", name="recip")
                    nc.vector.reciprocal(recip, av[h][64:65, :])
                    bc = work.tile([64, 512], f32, tag="bc", name="bc")
                    nc.gpsimd.partition_broadcast(bc, recip)
                    nc.vector.tensor_mul(
                        aoT[ts(h, 64), ts(qi, 512)], av[h][0:64, :], bc)
            # tail: projection of the final q-chunk
            for tl in range(4):
                for j in range(2):
                    emit_proj_chunk(7 * 4 + tl, j)

    nc.compile()
    _NC_CACHE[(reps, trace_sim)] = nc
    return nc


def make_in_maps(x, W_qkv, b_qkv, W_proj):
    x2 = np.asarray(x, dtype=np.float32).reshape(N, DIM)
    xTv = np.ascontiguousarray(x2.T.astype(np.float16))
    W_qkv = np.asarray(W_qkv, dtype=np.float32)
    W16 = W_qkv.astype(np.float16)
    b_qkv = np.asarray(b_qkv, dtype=np.float32)
    Wp16 = np.asarray(W_proj, dtype=np.float32).astype(np.float16)
    maps = []
    for m in range(NUM_CORES):
        h0 = m * DPC
        maps.append({
            "xT": xTv,
            "wq": np.ascontiguousarray(W16[:, h0:h0 + DPC]),
            "wk": np.ascontiguousarray(W16[:, DIM + h0:DIM + h0 + DPC]),
            "wv": np.ascontiguousarray(W16[:, 2 * DIM + h0:2 * DIM + h0 + DPC]),
            "wp": np.ascontiguousarray(Wp16[h0:h0 + DPC, :]),
            "bq": np.ascontiguousarray(b_qkv[h0:h0 + DPC].reshape(DPC, 1)),
            "bk": np.ascontiguousarray(
                b_qkv[DIM + h0:DIM + h0 + DPC].reshape(DPC, 1)),
            "ones": np.ones((1, 1), dtype=np.float16),
        })
    return maps


def kernel(x, W_qkv, b_qkv, W_proj, b_proj, _reps=1):
    from concourse.bass_utils import run_bass_kernel_spmd

    nc = build_nc(_reps)
    maps = make_in_maps(x, W_qkv, b_qkv, W_proj)
    res = run_bass_kernel_spmd(nc, maps, list(range(NUM_CORES)))
    total = np.zeros((N, DIM), dtype=np.float32)
    for r in res.results:
        total += r["out"].astype(np.float32)
    # bias corrections done on host: b_proj, plus bv @ W_proj (softmax rows
    # sum to 1, so the V-bias adds the constant row bv @ Wp to attn @ Wp)
    b_qkv = np.asarray(b_qkv, dtype=np.float32)
    bv = b_qkv[2 * DIM:3 * DIM]
    corr = bv @ np.asarray(W_proj, dtype=np.float32) + np.asarray(
        b_proj, dtype=np.float32)
    total = total + corr[None, :]
    return total.reshape(1, N, DIM).astype(np.float32)


# revision 5
# speedup vs baseline: 2.3749x; 2.3749x over previous
"""Trainium2 Bass kernel for multi-head attention (dense_transformer).

Full module: qkv = x @ W_qkv + b_qkv; multi-head attention (16 heads, d=64,
N=4096); out = attn @ W_proj + b_proj.

Sharding: tensor-parallel over heads - 2 heads per core on 8 cores. Each core
receives full x (pre-transposed on host to [C, N]) plus its head-slices of the
weights, computes its heads' attention and a partial output projection; the
host sums the 8 fp16 partials in f32 and adds b_proj plus the bv@Wp
correction (softmax rows sum to 1, so the V-bias contribution to the output
is the constant row bv @ Wp - computed on host, never on device).

v2 structure (per core, matmul operands fp16, PSUM f32):
  A) Q^T,K^T [128, 4096] = W^T @ x^T accumulated over C chunks, bias added on
     eviction. V^T likewise, PE-transposed to natural [tok, d] with a ones
     column per head: [V_h0 | 1 | V_h1 | 1] (cols 64/129 = denominator trick).
  B) per (q-chunk 512, k-chunk 128):
     - scores: TWO ROW-TILED matmuls (K=64 each, heads at PE rows 0-63 /
       64-127) run concurrently -> s [128, 1024] f32 (2 PSUM banks).
     - exp: split across engines. Most k-chunks: ACT exp (scale=1/8) PSUM ->
       fp16 SBUF. A spaced subset: custom 2-pass DVE op computing
       exp(s/8) = (T3(s/512))^64 (Taylor-3 + 6 squarings, max rel err 1.2e-4)
       so the Vector engine carries part of the 33.5M-element exp load that
       otherwise serializes on the Scalar engine (1 elem/lane/cycle).
     - AV: 2 matmuls (M=65, [V_h|1] stationary) accumulate av_h [65, 512];
       row 64 = softmax denominator. AV emission is deferred by a per-tile
       lag so slow DVE-exp tiles don't stall the PE.
     - normalize: DVE reciprocal of row 64 + GPSIMD partition broadcast +
       DVE mul into packed aoT [128, 4096] fp16 (h0 rows 0-63, h1 64-127).
  C) proj (lagged one q-chunk, spread through the ki loop): single matmul
     per chunk, K=128 contracts both heads at once: out[tok, C-half] =
     aoT_chunk^T @ Wp; evicted to fp16 and DMAd out.
"""

import numpy as np
from contextlib import ExitStack

NUM_CORES = 8
DIM = 1024
NUM_HEADS = 16
HDIM = 64
N = 4096
HPC = NUM_HEADS // NUM_CORES   # heads per core = 2
DPC = HPC * HDIM               # head dims per core = 128

# k-chunks (of 32 per q-chunk) whose exp runs on the DVE instead of ACT.
DVE_KIS = frozenset((14,))  # bisect: one DVE tile
LAG_ACT = 2   # AV follows scores by this many slots (ACT-exp tiles)
LAG_DVE = 2   # and for DVE-exp tiles (2-pass exp has ~2.3us latency)

_NC_CACHE = {}
_DVE_OPS = {}


def _register_dve_exp_ops():
    """Register the two custom DVE ops for exp(s/8) = (T3(s/512))^64.

    P1: x = s*C0; t = 1 + x + x^2/2 + x^3/6 (Horner); out = t^2   (8 stages)
    P2: out = in^32 (5 squarings)                                  (5 stages)
    """
    if _DVE_OPS:
        return _DVE_OPS
    from concourse.dve_spec import Spec, Src0, C0, C1, C2, One, sq, lower
    from concourse import dve_ops as dvo
    from concourse.dve_uop import DveOpSpec

    def _p1_ref(in0, in1, s0, s1, imm2):
        x = (np.asarray(in0, np.float32) * np.float32(s0)).astype(np.float32)
        h = (x * np.float32(s1)).astype(np.float32)
        h = (h + np.float32(imm2)).astype(np.float32)
        h = (h * x).astype(np.float32)
        h = (h + np.float32(1.0)).astype(np.float32)
        h = (h * x).astype(np.float32)
        h = (h + np.float32(1.0)).astype(np.float32)
        return (h * h).astype(np.float32)

    def _p2_ref(in0, in1, s0, s1, imm2):
        x = np.asarray(in0, np.float32)
        for _ in range(5):
            x = (x * x).astype(np.float32)
        return x

    x = Src0 * C0
    h = x * C1
    h = h + C2
    h = h * x
    h = h + One
    h = h * x
    h = h + One
    spec1 = Spec(body=sq(h), reference=_p1_ref)
    spec2 = Spec(body=sq(sq(sq(sq(sq(Src0))))), reference=_p2_ref)

    for name, spec in (("EXP_P1_ANT", spec1), ("EXP_P2_ANT", spec2)):
        if name in dvo._SUB_OPCODE_FOR_NAME:
            continue
        row = dvo._CUSTOM_DVE_ROW_BASE + len(dvo.OPS)
        assert row < 0x20
        shas = {}
        for ver in ("v3", "v4"):
            ds = DveOpSpec(name=name, opcode=row, uops=lower(spec, ver=ver),
                           rd1_en=False)
            shas[ver] = ds.sha(ver)
        op = dvo.DveOp(name, spec, subdim=False, uops_sha=shas)
        dvo.OPS.append(op)
        dvo._SUB_OPCODE_FOR_NAME[name] = row
        dvo.CUSTOM_DVE_SPECS[name] = spec
        _DVE_OPS[name] = op
    return _DVE_OPS


def build_nc(reps=1, trace_sim=False):
    if (reps, trace_sim) in _NC_CACHE:
        return _NC_CACHE[(reps, trace_sim)]

    import concourse.bass as bass
    import concourse.mybir as mybir
    import concourse.tile as tile
    from concourse import bacc
    from concourse.masks import make_identity

    ops = _register_dve_exp_ops()
    EXP_P1, EXP_P2 = ops["EXP_P1_ANT"], ops["EXP_P2_ANT"]

    f32 = mybir.dt.float32
    fp16 = mybir.dt.float16
    AF = mybir.ActivationFunctionType
    ts = bass.ts

    nc = bacc.Bacc(trn_type="TRN2", target_bir_lowering=False, debug=False)
    xT = nc.dram_tensor("xT", [DIM, N], fp16, kind="ExternalInput").ap()
    wq = nc.dram_tensor("wq", [DIM, DPC], fp16, kind="ExternalInput").ap()
    wk = nc.dram_tensor("wk", [DIM, DPC], fp16, kind="ExternalInput").ap()
    wv = nc.dram_tensor("wv", [DIM, DPC], fp16, kind="ExternalInput").ap()
    wp = nc.dram_tensor("wp", [DPC, DIM], fp16, kind="ExternalInput").ap()
    bq = nc.dram_tensor("bq", [DPC, 1], f32, kind="ExternalInput").ap()
    bk = nc.dram_tensor("bk", [DPC, 1], f32, kind="ExternalInput").ap()
    ones = nc.dram_tensor("ones", [1, 1], fp16, kind="ExternalInput").ap()
    out = nc.dram_tensor("out", [N, DIM], fp16, kind="ExternalOutput").ap()

    with tile.TileContext(nc, trace_sim=trace_sim) as tc, ExitStack() as ctx:
        singles = ctx.enter_context(tc.tile_pool(name="singles", bufs=1))
        psum = ctx.enter_context(tc.tile_pool(name="ps", bufs=2, space="PSUM"))
        xpool = ctx.enter_context(tc.tile_pool(name="xp", bufs=2))
        work = ctx.enter_context(tc.tile_pool(name="work", bufs=2))
        ppool = ctx.enter_context(tc.tile_pool(name="pp", bufs=3))
        opool = ctx.enter_context(tc.tile_pool(name="op", bufs=3))

        ident = singles.tile([128, 128], f32, tag="ident")
        make_identity(nc, ident)

        wq_sb = singles.tile([128, 8, DPC], fp16, tag="wq")
        wk_sb = singles.tile([128, 8, DPC], fp16, tag="wk")
        wv_sb = singles.tile([128, 8, DPC], fp16, tag="wv")
        nc.sync.dma_start(out=wq_sb, in_=wq.rearrange("(c p) m -> p c m", p=128))
        nc.sync.dma_start(out=wk_sb, in_=wk.rearrange("(c p) m -> p c m", p=128))
        nc.sync.dma_start(out=wv_sb, in_=wv.rearrange("(c p) m -> p c m", p=128))
        wp_sb = singles.tile([128, DIM], fp16, tag="wp")
        nc.sync.dma_start(out=wp_sb, in_=wp)
        bq_sb = singles.tile([DPC, 1], f32, tag="bq")
        bk_sb = singles.tile([DPC, 1], f32, tag="bk")
        nc.sync.dma_start(out=bq_sb, in_=bq)
        nc.sync.dma_start(out=bk_sb, in_=bk)

        qT = singles.tile([128, N], fp16, tag="qT")
        kT = singles.tile([128, N], fp16, tag="kT")
        aoT = singles.tile([128, N], fp16, tag="aoT")
        # V natural layout + ones column per head: [.., t, 0:64]=V_h0,
        # [.., t, 64]=1, [.., t, 65:129]=V_h1, [.., t, 129]=1
        v_nat = singles.tile([128, 32, 130], fp16, tag="vnat")
        nc.sync.dma_start(out=v_nat[:, :, 64:65], in_=ones.to_broadcast((128, 32, 1)))
        nc.sync.dma_start(out=v_nat[:, :, 129:130], in_=ones.to_broadcast((128, 32, 1)))

        for _rep in range(reps):
            # ---------------- Phase A: QKV projection ----------------
            for qt in range(4):
                xt = [xpool.tile([128, 1024], fp16, tag=f"x{c}", name=f"x{c}") for c in range(8)]
                for c in range(8):
                    nc.sync.dma_start(out=xt[c], in_=xT[ts(c, 128), ts(qt, 1024)])
                for nl in range(2):
                    n = qt * 2 + nl
                    # K and V first: attention waits on full K/V, while Q
                    # chunks are consumed per q-tile
                    acc = psum.tile([128, 512], f32, tag="pj", name="kacc", bufs=2)
                    for c in range(8):
                        nc.tensor.matmul(
                            acc, wk_sb[:, c, :], xt[c][:, ts(nl, 512)],
                            start=(c == 0), stop=(c == 7),
                        )
                    nc.vector.tensor_scalar_add(kT[:, ts(n, 512)], acc, bk_sb)
                    vacc = psum.tile([128, 512], f32, tag="pj", name="vacc", bufs=2)
                    for c in range(8):
                        nc.tensor.matmul(
                            vacc, wv_sb[:, c, :], xt[c][:, ts(nl, 512)],
                            start=(c == 0), stop=(c == 7),
                        )
                    vst = work.tile([128, 512], f32, tag="vst")
                    nc.scalar.copy(vst, vacc)
                    tpb = psum.tile([128, 512], f32, tag="s", name="tpb", bufs=2)
                    for tl in range(4):
                        nc.tensor.transpose(
                            tpb[:, ts(tl, 128)], vst[:, ts(tl, 128)], ident)
                    nc.vector.tensor_copy(
                        out=v_nat[:, ts(n, 4), 0:130]
                        .rearrange("p t (g d) -> p t g d", d=65)[:, :, :, 0:64],
                        in_=tpb.rearrange("p (t g d) -> p t g d", g=2, d=64),
                    )
                    qacc = psum.tile([128, 512], f32, tag="s", name="qacc", bufs=2)
                    for c in range(8):
                        nc.tensor.matmul(
                            qacc, wq_sb[:, c, :], xt[c][:, ts(nl, 512)],
                            start=(c == 0), stop=(c == 7),
                        )
                    nc.vector.tensor_scalar_add(qT[:, ts(n, 512)], qacc, bq_sb)

            # ---------------- Phase B: attention + lagged projection ------
            def emit_proj_chunk(t, j):
                pp = psum.tile([128, 512], f32, tag="pj", name="pp", bufs=2)
                nc.tensor.matmul(
                    pp, aoT[:, ts(t, 128)], wp_sb[:, ts(j, 512)],
                    start=True, stop=True,
                )
                ot = opool.tile([128, 512], fp16, tag="ot")
                nc.vector.tensor_copy(ot, pp)
                nc.sync.dma_start(out=out[ts(t, 128), ts(j, 512)], in_=ot)

            for qi in range(8):
                # proj tasks for the previous q-chunk, spread across this
                # q-chunk's slot loop so they fill PE slack
                proj_tasks = (
                    [((qi - 1) * 4 + tl, j) for tl in range(4) for j in range(2)]
                    if qi >= 1 else []
                )
                av = [
                    psum.tile([65, 512], f32, tag="av0", name="av0", bufs=1),
                    psum.tile([65, 512], f32, tag="av1", name="av1", bufs=1),
                ]
                pending = []  # (ki, p_tile, due_slot)
                n_av = 0
                for t in range(32 + LAG_DVE + 1):
                    if t < 32:
                        ki = t
                        s = psum.tile([128, 1024], f32, tag="s", name="s", bufs=2)
                        # two row-tiled matmuls: head0 on PE rows 0-63,
                        # head1 on rows 64-127 - run concurrently
                        nc.tensor.matmul(
                            s[:, 0:512], kT[0:64, ts(ki, 128)], qT[0:64, ts(qi, 512)],
                            start=True, stop=True,
                        )
                        nc.tensor.matmul(
                            s[:, 512:1024], kT[64:128, ts(ki, 128)],
                            qT[64:128, ts(qi, 512)],
                            start=True, stop=True,
                        )
                        p = ppool.tile([128, 1024], fp16, tag="p")
                        if ki in DVE_KIS:
                            mid = work.tile([128, 1024], f32, tag="mid", bufs=2)
                            nc.vector._custom_dve(
                                EXP_P1, out=mid, in0=s,
                                s0=1.0 / 512.0, s1=1.0 / 6.0, imm2=0.5)
                            nc.vector._custom_dve(EXP_P2, out=p, in0=mid)
                            due = t + LAG_DVE
                        else:
                            nc.scalar.activation(p, s, AF.Exp, scale=0.125)
                            due = t + LAG_ACT
                        pending.append((ki, p, due))
                    ready = [e for e in pending if e[2] <= t]
                    for e in ready:
                        pending.remove(e)
                        ki, p, _ = e
                        nc.tensor.matmul(
                            av[0], v_nat[:, ki, 0:65], p[:, 0:512],
                            start=(n_av == 0), stop=(n_av == 31),
                        )
                        nc.tensor.matmul(
                            av[1], v_nat[:, ki, 65:130], p[:, 512:1024],
                            start=(n_av == 0), stop=(n_av == 31),
                        )
                        n_av += 1
                    if t % 4 == 2 and proj_tasks:
                        emit_proj_chunk(*proj_tasks.pop(0))
                assert n_av == 32 and not pending
                for h in range(2):
                    recip = work.tile([1, 512], f32, tag="recip", name="recip")
                    nc.vector.reciprocal(recip, av[h][64:65, :])
                    bc = work.tile([64, 512], f32, tag="bc", name="bc")
                    nc.gpsimd.partition_broadcast(bc, recip)
                    nc.vector.tensor_mul(
                        aoT[ts(h, 64), ts(qi, 512)], av[h][0:64, :], bc)
            # tail: projection of the final q-chunk
            for tl in range(4):
                for j in range(2):
                    emit_proj_chunk(7 * 4 + tl, j)

    nc.compile()
    _NC_CACHE[(reps, trace_sim)] = nc
    return nc


def make_in_maps(x, W_qkv, b_qkv, W_proj):
    x2 = np.asarray(x, dtype=np.float32).reshape(N, DIM)
    xTv = np.ascontiguousarray(x2.T.astype(np.float16))
    W_qkv = np.asarray(W_qkv, dtype=np.float32)
    W16 = W_qkv.astype(np.float16)
    b_qkv = np.asarray(b_qkv, dtype=np.float32)
    Wp16 = np.asarray(W_proj, dtype=np.float32).astype(np.float16)
    maps = []
    for m in range(NUM_CORES):
        h0 = m * DPC
        maps.append({
            "xT": xTv,
            "wq": np.ascontiguousarray(W16[:, h0:h0 + DPC]),
            "wk": np.ascontiguousarray(W16[:, DIM + h0:DIM + h0 + DPC]),
            "wv": np.ascontiguousarray(W16[:, 2 * DIM + h0:2 * DIM + h0 + DPC]),
            "wp": np.ascontiguousarray(Wp16[h0:h0 + DPC, :]),
            "bq": np.ascontiguousarray(b_qkv[h0:h0 + DPC].reshape(DPC, 1)),
            "bk": np.ascontiguousarray(
                b_qkv[DIM + h0:DIM + h0 + DPC].reshape(DPC, 1)),
            "ones": np.ones((1, 1), dtype=np.float16),
        })
    return maps


def kernel(x, W_qkv, b_qkv, W_proj, b_proj, _reps=1):
    from concourse.bass_utils import run_bass_kernel_spmd

    nc = build_nc(_reps)
    maps = make_in_maps(x, W_qkv, b_qkv, W_proj)
    res = run_bass_kernel_spmd(nc, maps, list(range(NUM_CORES)))
    total = np.zeros((N, DIM), dtype=np.float32)
    for r in res.results:
        total += r["out"].astype(np.float32)
    # bias corrections done on host: b_proj, plus bv @ W_proj (softmax rows
    # sum to 1, so the V-bias adds the constant row bv @ Wp to attn @ Wp)
    b_qkv = np.asarray(b_qkv, dtype=np.float32)
    bv = b_qkv[2 * DIM:3 * DIM]
    corr = bv @ np.asarray(W_proj, dtype=np.float32) + np.asarray(
        b_proj, dtype=np.float32)
    total = total + corr[None, :]
    return total.reshape(1, N, DIM).astype(np.float32)


# revision 6
# speedup vs baseline: 2.8499x; 1.2000x over previous
"""Trainium2 Bass kernel for multi-head attention (dense_transformer).

Full module: qkv = x @ W_qkv + b_qkv; multi-head attention (16 heads, d=64,
N=4096); out = attn @ W_proj + b_proj.

Sharding: tensor-parallel over heads - 2 heads per core on 8 cores. Each core
receives full x (pre-transposed on host to [C, N]) plus its head-slices of the
weights, computes its heads' attention and a partial output projection; the
host sums the 8 fp16 partials in f32 and adds b_proj plus the bv@Wp
correction (softmax rows sum to 1, so the V-bias contribution to the output
is the constant row bv @ Wp - computed on host, never on device).

v2 structure (per core, matmul operands fp16, PSUM f32):
  A) Q^T,K^T [128, 4096] = W^T @ x^T accumulated over C chunks, bias added on
     eviction. V^T likewise, PE-transposed to natural [tok, d] with a ones
     column per head: [V_h0 | 1 | V_h1 | 1] (cols 64/129 = denominator trick).
  B) per (q-chunk 512, k-chunk 128):
     - scores: TWO ROW-TILED matmuls (K=64 each, heads at PE rows 0-63 /
       64-127) run concurrently -> s [128, 1024] f32 (2 PSUM banks).
     - exp: split across engines. Most k-chunks: ACT exp (scale=1/8) PSUM ->
       fp16 SBUF. A spaced subset: custom 2-pass DVE op computing
       exp(s/8) = (T3(s/512))^64 (Taylor-3 + 6 squarings, max rel err 1.2e-4)
       so the Vector engine carries part of the 33.5M-element exp load that
       otherwise serializes on the Scalar engine (1 elem/lane/cycle).
     - AV: 2 matmuls (M=65, [V_h|1] stationary) accumulate av_h [65, 512];
       row 64 = softmax denominator. AV emission is deferred by a per-tile
       lag so slow DVE-exp tiles don't stall the PE.
     - normalize: DVE reciprocal of row 64 + GPSIMD partition broadcast +
       DVE mul into packed aoT [128, 4096] fp16 (h0 rows 0-63, h1 64-127).
  C) proj (lagged one q-chunk, spread through the ki loop): single matmul
     per chunk, K=128 contracts both heads at once: out[tok, C-half] =
     aoT_chunk^T @ Wp; evicted to fp16 and DMAd out.
"""

import numpy as np
from contextlib import ExitStack

NUM_CORES = 8
DIM = 1024
NUM_HEADS = 16
HDIM = 64
N = 4096
HPC = NUM_HEADS // NUM_CORES   # heads per core = 2
DPC = HPC * HDIM               # head dims per core = 128

# k-chunks (of 32 per q-chunk) whose exp runs on the DVE instead of ACT.
DVE_KIS = frozenset((2, 5, 8, 11, 14, 17, 20, 23, 26))
LAG_ACT = 2   # AV follows scores by this many slots (ACT-exp tiles)
LAG_DVE = 2   # and for DVE-exp tiles (2-pass exp has ~2.3us latency)

_NC_CACHE = {}
_DVE_OPS = {}


def _register_dve_exp_ops():
    """Register the two custom DVE ops for exp(s/8) = (T3(s/512))^64.

    P1: x = s*C0; t = 1 + x + x^2/2 + x^3/6 (Horner); out = t^2   (8 stages)
    P2: out = in^32 (5 squarings)                                  (5 stages)
    """
    if _DVE_OPS:
        return _DVE_OPS
    from concourse.dve_spec import Spec, Src0, C0, C1, C2, One, sq, lower
    from concourse import dve_ops as dvo
    from concourse.dve_uop import DveOpSpec

    def _p1_ref(in0, in1, s0, s1, imm2):
        x = (np.asarray(in0, np.float32) * np.float32(s0)).astype(np.float32)
        h = (x * np.float32(s1)).astype(np.float32)
        h = (h + np.float32(imm2)).astype(np.float32)
        h = (h * x).astype(np.float32)
        h = (h + np.float32(1.0)).astype(np.float32)
        h = (h * x).astype(np.float32)
        h = (h + np.float32(1.0)).astype(np.float32)
        return (h * h).astype(np.float32)

    def _p2_ref(in0, in1, s0, s1, imm2):
        x = np.asarray(in0, np.float32)
        for _ in range(5):
            x = (x * x).astype(np.float32)
        return x

    x = Src0 * C0
    h = x * C1
    h = h + C2
    h = h * x
    h = h + One
    h = h * x
    h = h + One
    spec1 = Spec(body=sq(h), reference=_p1_ref)
    spec2 = Spec(body=sq(sq(sq(sq(sq(Src0))))), reference=_p2_ref)

    for name, spec in (("EXP_P1_ANT", spec1), ("EXP_P2_ANT", spec2)):
        if name in dvo._SUB_OPCODE_FOR_NAME:
            continue
        row = dvo._CUSTOM_DVE_ROW_BASE + len(dvo.OPS)
        assert row < 0x20
        shas = {}
        for ver in ("v3", "v4"):
            ds = DveOpSpec(name=name, opcode=row, uops=lower(spec, ver=ver),
                           rd1_en=False)
            shas[ver] = ds.sha(ver)
        op = dvo.DveOp(name, spec, subdim=False, uops_sha=shas)
        dvo.OPS.append(op)
        dvo._SUB_OPCODE_FOR_NAME[name] = row
        dvo.CUSTOM_DVE_SPECS[name] = spec
        _DVE_OPS[name] = op
    return _DVE_OPS


def build_nc(reps=1, trace_sim=False):
    if (reps, trace_sim) in _NC_CACHE:
        return _NC_CACHE[(reps, trace_sim)]

    import concourse.bass as bass
    import concourse.mybir as mybir
    import concourse.tile as tile
    from concourse import bacc
    from concourse.masks import make_identity

    ops = _register_dve_exp_ops()
    EXP_P1, EXP_P2 = ops["EXP_P1_ANT"], ops["EXP_P2_ANT"]

    f32 = mybir.dt.float32
    fp16 = mybir.dt.float16
    AF = mybir.ActivationFunctionType
    ts = bass.ts

    nc = bacc.Bacc(trn_type="TRN2", target_bir_lowering=False, debug=False)
    xT = nc.dram_tensor("xT", [DIM, N], fp16, kind="ExternalInput").ap()
    wq = nc.dram_tensor("wq", [DIM, DPC], fp16, kind="ExternalInput").ap()
    wk = nc.dram_tensor("wk", [DIM, DPC], fp16, kind="ExternalInput").ap()
    wv = nc.dram_tensor("wv", [DIM, DPC], fp16, kind="ExternalInput").ap()
    wp = nc.dram_tensor("wp", [DPC, DIM], fp16, kind="ExternalInput").ap()
    bq = nc.dram_tensor("bq", [DPC, 1], f32, kind="ExternalInput").ap()
    bk = nc.dram_tensor("bk", [DPC, 1], f32, kind="ExternalInput").ap()
    ones = nc.dram_tensor("ones", [1, 1], fp16, kind="ExternalInput").ap()
    out = nc.dram_tensor("out", [N, DIM], fp16, kind="ExternalOutput").ap()

    with tile.TileContext(nc, trace_sim=trace_sim) as tc, ExitStack() as ctx:
        singles = ctx.enter_context(tc.tile_pool(name="singles", bufs=1))
        psum = ctx.enter_context(tc.tile_pool(name="ps", bufs=2, space="PSUM"))
        xpool = ctx.enter_context(tc.tile_pool(name="xp", bufs=2))
        work = ctx.enter_context(tc.tile_pool(name="work", bufs=2))
        ppool = ctx.enter_context(tc.tile_pool(name="pp", bufs=3))
        opool = ctx.enter_context(tc.tile_pool(name="op", bufs=3))

        ident = singles.tile([128, 128], f32, tag="ident")
        make_identity(nc, ident)

        wq_sb = singles.tile([128, 8, DPC], fp16, tag="wq")
        wk_sb = singles.tile([128, 8, DPC], fp16, tag="wk")
        wv_sb = singles.tile([128, 8, DPC], fp16, tag="wv")
        nc.sync.dma_start(out=wq_sb, in_=wq.rearrange("(c p) m -> p c m", p=128))
        nc.sync.dma_start(out=wk_sb, in_=wk.rearrange("(c p) m -> p c m", p=128))
        nc.sync.dma_start(out=wv_sb, in_=wv.rearrange("(c p) m -> p c m", p=128))
        wp_sb = singles.tile([128, DIM], fp16, tag="wp")
        nc.sync.dma_start(out=wp_sb, in_=wp)
        bq_sb = singles.tile([DPC, 1], f32, tag="bq")
        bk_sb = singles.tile([DPC, 1], f32, tag="bk")
        nc.sync.dma_start(out=bq_sb, in_=bq)
        nc.sync.dma_start(out=bk_sb, in_=bk)

        qT = singles.tile([128, N], fp16, tag="qT")
        kT = singles.tile([128, N], fp16, tag="kT")
        aoT = singles.tile([128, N], fp16, tag="aoT")
        # V natural layout + ones column per head: [.., t, 0:64]=V_h0,
        # [.., t, 64]=1, [.., t, 65:129]=V_h1, [.., t, 129]=1
        v_nat = singles.tile([128, 32, 130], fp16, tag="vnat")
        nc.sync.dma_start(out=v_nat[:, :, 64:65], in_=ones.to_broadcast((128, 32, 1)))
        nc.sync.dma_start(out=v_nat[:, :, 129:130], in_=ones.to_broadcast((128, 32, 1)))

        for _rep in range(reps):
            # ---------------- Phase A: QKV projection ----------------
            for qt in range(4):
                xt = [xpool.tile([128, 1024], fp16, tag=f"x{c}", name=f"x{c}") for c in range(8)]
                for c in range(8):
                    nc.sync.dma_start(out=xt[c], in_=xT[ts(c, 128), ts(qt, 1024)])
                for nl in range(2):
                    n = qt * 2 + nl
                    # K and V first: attention waits on full K/V, while Q
                    # chunks are consumed per q-tile
                    acc = psum.tile([128, 512], f32, tag="pj", name="kacc", bufs=2)
                    for c in range(8):
                        nc.tensor.matmul(
                            acc, wk_sb[:, c, :], xt[c][:, ts(nl, 512)],
                            start=(c == 0), stop=(c == 7),
                        )
                    nc.vector.tensor_scalar_add(kT[:, ts(n, 512)], acc, bk_sb)
                    vacc = psum.tile([128, 512], f32, tag="pj", name="vacc", bufs=2)
                    for c in range(8):
                        nc.tensor.matmul(
                            vacc, wv_sb[:, c, :], xt[c][:, ts(nl, 512)],
                            start=(c == 0), stop=(c == 7),
                        )
                    vst = work.tile([128, 512], f32, tag="vst")
                    nc.scalar.copy(vst, vacc)
                    tpb = psum.tile([128, 512], f32, tag="s", name="tpb", bufs=2)
                    for tl in range(4):
                        nc.tensor.transpose(
                            tpb[:, ts(tl, 128)], vst[:, ts(tl, 128)], ident)
                    nc.vector.tensor_copy(
                        out=v_nat[:, ts(n, 4), 0:130]
                        .rearrange("p t (g d) -> p t g d", d=65)[:, :, :, 0:64],
                        in_=tpb.rearrange("p (t g d) -> p t g d", g=2, d=64),
                    )
                    qacc = psum.tile([128, 512], f32, tag="s", name="qacc", bufs=2)
                    for c in range(8):
                        nc.tensor.matmul(
                            qacc, wq_sb[:, c, :], xt[c][:, ts(nl, 512)],
                            start=(c == 0), stop=(c == 7),
                        )
                    nc.vector.tensor_scalar_add(qT[:, ts(n, 512)], qacc, bq_sb)

            # ---------------- Phase B: attention + lagged projection ------
            def emit_proj_chunk(t, j):
                pp = psum.tile([128, 512], f32, tag="pj", name="pp", bufs=2)
                nc.tensor.matmul(
                    pp, aoT[:, ts(t, 128)], wp_sb[:, ts(j, 512)],
                    start=True, stop=True,
                )
                ot = opool.tile([128, 512], fp16, tag="ot")
                nc.vector.tensor_copy(ot, pp)
                nc.sync.dma_start(out=out[ts(t, 128), ts(j, 512)], in_=ot)

            for qi in range(8):
                # proj tasks for the previous q-chunk, spread across this
                # q-chunk's slot loop so they fill PE slack
                proj_tasks = (
                    [((qi - 1) * 4 + tl, j) for tl in range(4) for j in range(2)]
                    if qi >= 1 else []
                )
                av = [
                    psum.tile([65, 512], f32, tag="av0", name="av0", bufs=1),
                    psum.tile([65, 512], f32, tag="av1", name="av1", bufs=1),
                ]
                pending = []  # (ki, p_tile, due_slot)
                n_av = 0
                for t in range(32 + LAG_DVE + 1):
                    if t < 32:
                        ki = t
                        s = psum.tile([128, 1024], f32, tag="s", name="s", bufs=2)
                        # two row-tiled matmuls: head0 on PE rows 0-63,
                        # head1 on rows 64-127 - run concurrently
                        nc.tensor.matmul(
                            s[:, 0:512], kT[0:64, ts(ki, 128)], qT[0:64, ts(qi, 512)],
                            start=True, stop=True,
                        )
                        nc.tensor.matmul(
                            s[:, 512:1024], kT[64:128, ts(ki, 128)],
                            qT[64:128, ts(qi, 512)],
                            start=True, stop=True,
                        )
                        p = ppool.tile([128, 1024], fp16, tag="p")
                        if ki in DVE_KIS:
                            mid = work.tile([128, 1024], f32, tag="mid", bufs=2)
                            nc.vector._custom_dve(
                                EXP_P1, out=mid, in0=s,
                                s0=1.0 / 512.0, s1=1.0 / 6.0, imm2=0.5)
                            nc.vector._custom_dve(EXP_P2, out=p, in0=mid)
                            due = t + LAG_DVE
                        else:
                            nc.scalar.activation(p, s, AF.Exp, scale=0.125)
                            due = t + LAG_ACT
                        pending.append((ki, p, due))
                    ready = [e for e in pending if e[2] <= t]
                    for e in ready:
                        pending.remove(e)
                        ki, p, _ = e
                        nc.tensor.matmul(
                            av[0], v_nat[:, ki, 0:65], p[:, 0:512],
                            start=(n_av == 0), stop=(n_av == 31),
                        )
                        nc.tensor.matmul(
                            av[1], v_nat[:, ki, 65:130], p[:, 512:1024],
                            start=(n_av == 0), stop=(n_av == 31),
                        )
                        n_av += 1
                    if t % 4 == 2 and proj_tasks:
                        emit_proj_chunk(*proj_tasks.pop(0))
                assert n_av == 32 and not pending
                for h in range(2):
                    recip = work.tile([1, 512], f32, tag="recip", name="recip")
                    nc.vector.reciprocal(recip, av[h][64:65, :])
                    bc = work.tile([64, 512], f32, tag="bc", name="bc")
                    nc.gpsimd.partition_broadcast(bc, recip)
                    nc.vector.tensor_mul(
                        aoT[ts(h, 64), ts(qi, 512)], av[h][0:64, :], bc)
            # tail: projection of the final q-chunk
            for tl in range(4):
                for j in range(2):
                    emit_proj_chunk(7 * 4 + tl, j)

    nc.compile()
    _NC_CACHE[(reps, trace_sim)] = nc
    return nc


def make_in_maps(x, W_qkv, b_qkv, W_proj):
    x2 = np.asarray(x, dtype=np.float32).reshape(N, DIM)
    xTv = np.ascontiguousarray(x2.T.astype(np.float16))
    W_qkv = np.asarray(W_qkv, dtype=np.float32)
    W16 = W_qkv.astype(np.float16)
    b_qkv = np.asarray(b_qkv, dtype=np.float32)
    Wp16 = np.asarray(W_proj, dtype=np.float32).astype(np.float16)
    maps = []
    for m in range(NUM_CORES):
        h0 = m * DPC
        maps.append({
            "xT": xTv,
            "wq": np.ascontiguousarray(W16[:, h0:h0 + DPC]),
            "wk": np.ascontiguousarray(W16[:, DIM + h0:DIM + h0 + DPC]),
            "wv": np.ascontiguousarray(W16[:, 2 * DIM + h0:2 * DIM + h0 + DPC]),
            "wp": np.ascontiguousarray(Wp16[h0:h0 + DPC, :]),
            "bq": np.ascontiguousarray(b_qkv[h0:h0 + DPC].reshape(DPC, 1)),
            "bk": np.ascontiguousarray(
                b_qkv[DIM + h0:DIM + h0 + DPC].reshape(DPC, 1)),
            "ones": np.ones((1, 1), dtype=np.float16),
        })
    return maps


def kernel(x, W_qkv, b_qkv, W_proj, b_proj, _reps=1):
    from concourse.bass_utils import run_bass_kernel_spmd

    nc = build_nc(_reps)
    maps = make_in_maps(x, W_qkv, b_qkv, W_proj)
    res = run_bass_kernel_spmd(nc, maps, list(range(NUM_CORES)))
    total = np.zeros((N, DIM), dtype=np.float32)
    for r in res.results:
        total += r["out"].astype(np.float32)
    # bias corrections done on host: b_proj, plus bv @ W_proj (softmax rows
    # sum to 1, so the V-bias adds the constant row bv @ Wp to attn @ Wp)
    b_qkv = np.asarray(b_qkv, dtype=np.float32)
    bv = b_qkv[2 * DIM:3 * DIM]
    corr = bv @ np.asarray(W_proj, dtype=np.float32) + np.asarray(
        b_proj, dtype=np.float32)
    total = total + corr[None, :]
    return total.reshape(1, N, DIM).astype(np.float32)
